# revision 62
# baseline (speedup 1.0000x reference)
"""Trainium2 Bass kernel for MoGNN forward (global mean-pool + linear).

The model's conv outputs are discarded; the result depends only on x:
    pooled[g] = mean over nodes n with batch[n] == g of x[n]   # [1024, 512]
    out = pooled @ W.T + b                                     # [1024, 7]

batch ids are sorted, so nodes of each graph are contiguous. We shard by
GRAPHS: core k owns graphs [128k, 128k+128) and exactly the contiguous row
range of x belonging to them (padded to a tile multiple). No collectives.

Per 128-node tile, on device:
  - DVE builds an exact one-hot matrix oh[n, g] = (batch_local[n] == g);
    one tensor_tensor(is_equal) per DMA chunk via step-0 broadcast APs.
  - PE matmul (fp16 in, fp32 PSUM accumulate, full rate at N=512) does
    psum[128 graphs, 512 feats] += oh.T @ x_tile.
Epilogue: PSUM -> SBUF with a per-graph 1/count scale (mean pool), 4x PE
transpose to feat-major (pipelined behind the sliced scale), then 4 fp32
matmuls with pooled.T stationary and the W chunk moving (N=7), bias added
via a partition-replicated tile; each core writes out[128, 7] and the host
concatenates to [1024, 7].

x is shipped as fp16 (11-bit effective mantissa; accumulation stays fp32 in
PSUM) — measured end-to-end relative error vs the fp32 reference ~2e-4,
comparable to the fp32r (tf32-like) matmul path while halving HBM traffic.
"""

import numpy as np

NCORES = 8
G = 1024            # total graphs
GPC = G // NCORES   # graphs per core = 128
F = 512             # feature dim
P = 128             # partition / node-tile size
CHUNK = 8           # node tiles per DMA chunk (1 MB fp16 transfers)

_compiled_cache = {}


def _chunk_plan(ntiles):
    """Chunk boundaries: small leading chunks so the PE pipeline starts early,
    CHUNK-tile steady state, and a small taper at the end so the PE finishes
    right behind the final DMA bytes."""
    head = [min(2, CHUNK), min(6, CHUNK)]
    tail = [min(2, CHUNK)]
    main_end = max(ntiles - sum(tail), 0)
    chunks = []
    t0 = 0
    for ramp in head:
        if t0 < main_end:
            clen = min(ramp, main_end - t0)
            chunks.append((t0, clen))
            t0 += clen
    while t0 < main_end:
        clen = min(CHUNK, main_end - t0)
        chunks.append((t0, clen))
        t0 += clen
    for ramp in tail:
        if t0 < ntiles:
            clen = min(ramp, ntiles - t0)
            chunks.append((t0, clen))
            t0 += clen
    while t0 < ntiles:
        clen = min(CHUNK, ntiles - t0)
        chunks.append((t0, clen))
        t0 += clen
    assert sum(c for _, c in chunks) == ntiles
    return chunks


def _build(ntiles):
    """Build + compile the per-core Bass kernel for a shard of `ntiles` node tiles."""
    from concourse import bacc, tile, mybir

    f32 = mybir.dt.float32
    f16 = mybir.dt.float16
    eq = mybir.AluOpType.is_equal
    mult = mybir.AluOpType.mult
    add = mybir.AluOpType.add

    nrows = ntiles * P
    chunks = _chunk_plan(ntiles)

    nc = bacc.Bacc(
        "TRN2",
        target_bir_lowering=False,
        debug=False,
        num_devices=NCORES,
    )

    # x shard laid out chunk-contiguous and partition-major inside each chunk:
    # for chunk (c0, clen), the DRAM block holds block[p, t, m] = x[(c0+t)*128+p, m]
    # so the whole chunk is one contiguous region and each partition reads one
    # contiguous multi-KB run
    x_d = nc.dram_tensor("xs", [nrows * F], f16, kind="ExternalInput")
    # constants packed into two tensors (one DMA each, on the scalar-engine
    # HWDGE ring so they don't block the x-chunk FIFO on the sync ring):
    # cp16 = [bl | iota], cp32 = [ident | wtr | b | icnt]
    # cp16 = [bl | iota | ident | wtr], cp32 = [b_replicated | icnt]
    cp16_d = nc.dram_tensor(
        "cp16", [P, ntiles + GPC + P + 28], f16, kind="ExternalInput"
    )
    cp32_d = nc.dram_tensor("cp32", [P, 8], f32, kind="ExternalInput")
    out_d = nc.dram_tensor("out", [GPC, 7], f32, kind="ExternalOutput")

    with tile.TileContext(nc) as tc:
        with (
            tc.tile_pool(name="const", bufs=1) as constp,
            tc.tile_pool(name="xin", bufs=4) as xp,
            tc.tile_pool(name="oh", bufs=4) as ohp,
            tc.tile_pool(name="acc", bufs=1, space="PSUM") as accp,
            tc.tile_pool(name="tps", bufs=2, space="PSUM") as tpsp,
            tc.tile_pool(name="outp", bufs=1, space="PSUM") as outpp,
            tc.tile_pool(name="sb", bufs=2) as sbp,
        ):
            cp16_t = constp.tile([P, ntiles + GPC + P + 28], f16)
            nc.scalar.dma_start(cp16_t[:], cp16_d.ap())
            cp32_t = constp.tile([P, 8], f32)
            nc.scalar.dma_start(cp32_t[:], cp32_d.ap())
            bl_t = cp16_t[:, 0:ntiles]
            iota_t = cp16_t[:, ntiles : ntiles + GPC]
            ident_t = cp16_t[:, ntiles + GPC : ntiles + GPC + P]
            wtr_t = cp16_t[:, ntiles + GPC + P : ntiles + GPC + P + 28]
            brep_t = cp32_t[:, 0:7]
            icnt_t = cp32_t[:, 7:8]

            acc = accp.tile([GPC, F], f32)
            x_flat = x_d.ap()

            iota_rep = iota_t.rearrange("p (a g) -> p a g", a=1)
            t = 0
            for c0, clen in chunks:
                xt = xp.tile([P, CHUNK, F], f16, tag="xt")
                chunk_ap = x_flat[c0 * P * F : (c0 + clen) * P * F].rearrange(
                    "(p t m) -> p t m", p=P, m=F
                )
                nc.sync.dma_start(xt[:, :clen, :], chunk_ap)
                # one-hot for the whole chunk in one DVE op via broadcast APs:
                # oh[p, n, g] = (iota[g] == bl[p, c0+n])
                oh = ohp.tile([P, CHUNK, GPC], f16, tag="oh")
                nc.vector.tensor_tensor(
                    oh[:, :clen, :],
                    iota_rep.broadcast_to([P, clen, GPC]),
                    bl_t[:, c0 : c0 + clen]
                    .rearrange("p (n a) -> p n a", a=1)
                    .broadcast_to([P, clen, GPC]),
                    op=eq,
                )
                for n in range(clen):
                    nc.tensor.matmul(
                        acc[:],
                        oh[:, n, :],
                        xt[:, n, :],
                        start=(t == 0),
                        stop=(t == ntiles - 1),
                    )
                    t += 1

            # pooled = acc * (1/count[g]) cast to fp16, sliced so the (fp16,
            # full-rate) transposes pipeline behind the scale copies; then the
            # classifier with pooled.T as stationary (moving is W [128, 7], N=7)
            pooled = sbp.tile([GPC, F], f16)
            ptall = sbp.tile([P, 4, P], f16)
            for j in range(4):
                sl = slice(j * P, (j + 1) * P)
                nc.vector.tensor_scalar(
                    pooled[:, sl], acc[:, sl], icnt_t, None, op0=mult
                )
                tp = tpsp.tile([P, P], f16)
                nc.tensor.transpose(tp[:], pooled[:, sl], ident_t)
                nc.vector.tensor_copy(ptall[:, j, :], tp[:])

            out_ps = outpp.tile([GPC, 7], f32)
            for j in range(4):
                nc.tensor.matmul(
                    out_ps[:],
                    ptall[:, j, :],
                    wtr_t[:, j * 7 : (j + 1) * 7],
                    start=(j == 0),
                    stop=(j == 3),
                )

            out_sb = sbp.tile([GPC, 7], f32)
            nc.vector.tensor_tensor(out_sb[:], out_ps[:], brep_t, op=add)
            nc.sync.dma_start(out_d.ap(), out_sb[:])

    nc.compile()
    return nc


def _get_compiled(ntiles):
    if ntiles not in _compiled_cache:
        _compiled_cache[ntiles] = _build(ntiles)
    return _compiled_cache[ntiles]


def _prep_in_maps(x16, batch, W, b, ntiles, bounds, inv_counts):
    cap = ntiles * P
    chunk_plan = _chunk_plan(ntiles)
    iota = np.tile(np.arange(GPC, dtype=np.float16)[None, :], (P, 1))
    # wtr[p, c*7+j] = W.T[c*128+p, j]
    wtr = np.ascontiguousarray(
        W.T.reshape(4, P, 7).transpose(1, 0, 2).reshape(P, 28)
    ).astype(np.float16)
    cp32_base = np.zeros((P, 8), dtype=np.float32)
    cp32_base[:, 0:7] = b.astype(np.float32)[None, :]

    in_maps = []
    for k in range(NCORES):
        lo, hi = int(bounds[k]), int(bounds[k + 1])
        n = hi - lo
        xs = np.zeros((cap, F), dtype=np.float16)
        xs[:n] = x16[lo:hi]
        # chunk-contiguous, partition-major within each chunk
        xs = xs.reshape(ntiles, P, F)
        parts = [
            np.ascontiguousarray(xs[c0 : c0 + clen].transpose(1, 0, 2)).reshape(-1)
            for c0, clen in chunk_plan
        ]
        xs = np.concatenate(parts)
        blv = np.full((cap,), -1.0, dtype=np.float16)
        blv[:n] = (batch[lo:hi] - GPC * k).astype(np.float16)
        cp16 = np.empty((P, ntiles + GPC + P + 28), dtype=np.float16)
        cp16[:, 0:ntiles] = blv.reshape(ntiles, P).T
        cp16[:, ntiles : ntiles + GPC] = iota
        cp16[:, ntiles + GPC : ntiles + GPC + P] = np.eye(P, dtype=np.float16)
        cp16[:, ntiles + GPC + P :] = wtr
        cp32 = cp32_base.copy()
        cp32[:, 7] = inv_counts[GPC * k : GPC * (k + 1)]
        in_maps.append({"xs": xs, "cp16": cp16, "cp32": cp32})
    return in_maps


_last_result = None  # test harness can read exec_time_ns / trace from here


def kernel(x, edge_index, edge_attr, batch_size, W, b):
    from concourse import bass_utils

    global _last_result

    x16 = np.asarray(x, dtype=np.float16)
    batch = np.asarray(batch_size).astype(np.int64)
    W = np.asarray(W, dtype=np.float32)
    b = np.asarray(b, dtype=np.float32)

    if batch.size > 1 and np.any(np.diff(batch) < 0):
        # contiguous-shard logic needs sorted ids; reordering nodes does not
        # change per-graph sums
        order = np.argsort(batch, kind="stable")
        batch = batch[order]
        x16 = x16[order]

    counts = np.bincount(batch, minlength=G)
    inv_counts = (1.0 / np.maximum(counts, 1)).astype(np.float32)
    bounds = np.searchsorted(batch, np.arange(0, G + 1, GPC))
    max_rows = int(np.diff(bounds).max())
    ntiles = max(-(-max_rows // P), 1)

    nc = _get_compiled(ntiles)
    in_maps = _prep_in_maps(x16, batch, W, b, ntiles, bounds, inv_counts)

    res = bass_utils.run_bass_kernel_spmd(
        nc, in_maps, core_ids=list(range(NCORES))
    )
    _last_result = res

    # each core returns out [128, 7] for its graphs; assemble [1024, 7]
    out = np.concatenate(
        [np.asarray(res.results[k]["out"]) for k in range(NCORES)], axis=0
    )
    return np.ascontiguousarray(out.astype(np.float32))


# revision 63
# speedup vs baseline: 1.0220x; 1.0220x over previous
"""Trainium2 Bass kernel for MoGNN forward (global mean-pool + linear).

The model's conv outputs are discarded; the result depends only on x:
    pooled[g] = mean over nodes n with batch[n] == g of x[n]   # [1024, 512]
    out = pooled @ W.T + b                                     # [1024, 7]

batch ids are sorted, so nodes of each graph are contiguous. We shard by
GRAPHS: core k owns graphs [128k, 128k+128) and exactly the contiguous row
range of x belonging to them (padded to a tile multiple). No collectives.

Per 128-node tile, on device:
  - DVE builds an exact one-hot matrix oh[n, g] = (batch_local[n] == g);
    one tensor_tensor(is_equal) per DMA chunk via step-0 broadcast APs.
  - PE matmul (fp16 in, fp32 PSUM accumulate, full rate at N=512) does
    psum[128 graphs, 512 feats] += oh.T @ x_tile.
Epilogue (all fp16 for full-rate PE): PSUM -> SBUF with a per-graph 1/count
scale (mean pool), 4x PE transpose to feat-major (pipelined behind the
sliced scale), then 4 fp16 matmuls with pooled.T stationary and the W chunk
moving (N=7, fp32 PSUM), bias added via a partition-replicated fp32 tile;
each core writes out[128, 7] and the host concatenates to [1024, 7].

x is shipped as fp16 (11-bit effective mantissa; accumulation stays fp32 in
PSUM) — measured end-to-end relative error vs the fp32 reference ~2e-4,
comparable to the fp32r (tf32-like) matmul path while halving HBM traffic.
"""

import numpy as np

NCORES = 8
G = 1024            # total graphs
GPC = G // NCORES   # graphs per core = 128
F = 512             # feature dim
P = 128             # partition / node-tile size
CHUNK = 8           # node tiles per DMA chunk (1 MB fp16 transfers)

_compiled_cache = {}


def _chunk_plan(ntiles):
    """Chunk boundaries: small leading chunks so the PE pipeline starts early,
    CHUNK-tile steady state, and a small taper at the end so the PE finishes
    right behind the final DMA bytes."""
    head = [min(2, CHUNK), min(6, CHUNK)]
    tail = [min(2, CHUNK)]
    main_end = max(ntiles - sum(tail), 0)
    chunks = []
    t0 = 0
    for ramp in head:
        if t0 < main_end:
            clen = min(ramp, main_end - t0)
            chunks.append((t0, clen))
            t0 += clen
    while t0 < main_end:
        clen = min(CHUNK, main_end - t0)
        chunks.append((t0, clen))
        t0 += clen
    for ramp in tail:
        if t0 < ntiles:
            clen = min(ramp, ntiles - t0)
            chunks.append((t0, clen))
            t0 += clen
    while t0 < ntiles:
        clen = min(CHUNK, ntiles - t0)
        chunks.append((t0, clen))
        t0 += clen
    assert sum(c for _, c in chunks) == ntiles
    return chunks


def _build(ntiles):
    """Build + compile the per-core Bass kernel for a shard of `ntiles` node tiles."""
    from concourse import bacc, tile, mybir

    f32 = mybir.dt.float32
    f16 = mybir.dt.float16
    eq = mybir.AluOpType.is_equal
    mult = mybir.AluOpType.mult
    add = mybir.AluOpType.add

    nrows = ntiles * P
    chunks = _chunk_plan(ntiles)

    nc = bacc.Bacc(
        "TRN2",
        target_bir_lowering=False,
        debug=False,
        num_devices=NCORES,
    )

    # x shard laid out chunk-contiguous and partition-major inside each chunk:
    # for chunk (c0, clen), the DRAM block holds block[p, t, m] = x[(c0+t)*128+p, m]
    # so the whole chunk is one contiguous region and each partition reads one
    # contiguous multi-KB run
    x_d = nc.dram_tensor("xs", [nrows * F], f16, kind="ExternalInput")
    # constants packed into two tensors (one DMA each, on the scalar-engine
    # HWDGE ring so they don't block the x-chunk FIFO on the sync ring):
    # cp16 = [bl | iota], cp32 = [ident | wtr | b | icnt]
    # cp16 = [bl | iota | ident | wtr], cp32 = [b_replicated | icnt]
    cp16_d = nc.dram_tensor(
        "cp16", [P, ntiles + GPC + P + 28], f16, kind="ExternalInput"
    )
    cp32_d = nc.dram_tensor("cp32", [P, 8], f32, kind="ExternalInput")
    out_d = nc.dram_tensor("out", [GPC, 7], f32, kind="ExternalOutput")

    with tile.TileContext(nc) as tc:
        with (
            tc.tile_pool(name="const", bufs=1) as constp,
            tc.tile_pool(name="xin", bufs=4) as xp,
            tc.tile_pool(name="oh", bufs=4) as ohp,
            tc.tile_pool(name="acc", bufs=1, space="PSUM") as accp,
            tc.tile_pool(name="tps", bufs=2, space="PSUM") as tpsp,
            tc.tile_pool(name="outp", bufs=1, space="PSUM") as outpp,
            tc.tile_pool(name="sb", bufs=2) as sbp,
        ):
            cp16_t = constp.tile([P, ntiles + GPC + P + 28], f16)
            nc.scalar.dma_start(cp16_t[:], cp16_d.ap())
            cp32_t = constp.tile([P, 8], f32)
            nc.scalar.dma_start(cp32_t[:], cp32_d.ap())
            bl_t = cp16_t[:, 0:ntiles]
            iota_t = cp16_t[:, ntiles : ntiles + GPC]
            ident_t = cp16_t[:, ntiles + GPC : ntiles + GPC + P]
            wtr_t = cp16_t[:, ntiles + GPC + P : ntiles + GPC + P + 28]
            brep_t = cp32_t[:, 0:7]
            icnt_t = cp32_t[:, 7:8]

            acc = accp.tile([GPC, F], f32)
            x_flat = x_d.ap()

            iota_rep = iota_t.rearrange("p (a g) -> p a g", a=1)
            t = 0
            for c0, clen in chunks:
                xt = xp.tile([P, CHUNK, F], f16, tag="xt")
                chunk_ap = x_flat[c0 * P * F : (c0 + clen) * P * F].rearrange(
                    "(p t m) -> p t m", p=P, m=F
                )
                nc.sync.dma_start(xt[:, :clen, :], chunk_ap)
                # one-hot for the whole chunk in one DVE op via broadcast APs:
                # oh[p, n, g] = (iota[g] == bl[p, c0+n])
                oh = ohp.tile([P, CHUNK, GPC], f16, tag="oh")
                nc.vector.tensor_tensor(
                    oh[:, :clen, :],
                    iota_rep.broadcast_to([P, clen, GPC]),
                    bl_t[:, c0 : c0 + clen]
                    .rearrange("p (n a) -> p n a", a=1)
                    .broadcast_to([P, clen, GPC]),
                    op=eq,
                )
                for n in range(clen):
                    nc.tensor.matmul(
                        acc[:],
                        oh[:, n, :],
                        xt[:, n, :],
                        start=(t == 0),
                        stop=(t == ntiles - 1),
                    )
                    t += 1

            # pooled = acc * (1/count[g]) cast to fp16, sliced so the (fp16,
            # full-rate) transposes pipeline behind the scale copies; then the
            # classifier with pooled.T as stationary (moving is W [128, 7], N=7)
            pooled = sbp.tile([GPC, F], f16)
            ptall = sbp.tile([P, 4, P], f16)
            for j in range(4):
                sl = slice(j * P, (j + 1) * P)
                nc.vector.tensor_scalar(
                    pooled[:, sl], acc[:, sl], icnt_t, None, op0=mult
                )
                tp = tpsp.tile([P, P], f16)
                nc.tensor.transpose(tp[:], pooled[:, sl], ident_t)
                nc.vector.tensor_copy(ptall[:, j, :], tp[:])

            out_ps = outpp.tile([GPC, 7], f32)
            for j in range(4):
                nc.tensor.matmul(
                    out_ps[:],
                    ptall[:, j, :],
                    wtr_t[:, j * 7 : (j + 1) * 7],
                    start=(j == 0),
                    stop=(j == 3),
                )

            out_sb = sbp.tile([GPC, 7], f32)
            nc.vector.tensor_tensor(out_sb[:], out_ps[:], brep_t, op=add)
            nc.sync.dma_start(out_d.ap(), out_sb[:])

    nc.compile()
    return nc


def _get_compiled(ntiles):
    if ntiles not in _compiled_cache:
        _compiled_cache[ntiles] = _build(ntiles)
    return _compiled_cache[ntiles]


def _prep_in_maps(x16, batch, W, b, ntiles, bounds, inv_counts):
    cap = ntiles * P
    chunk_plan = _chunk_plan(ntiles)
    iota = np.tile(np.arange(GPC, dtype=np.float16)[None, :], (P, 1))
    # wtr[p, c*7+j] = W.T[c*128+p, j]
    wtr = np.ascontiguousarray(
        W.T.reshape(4, P, 7).transpose(1, 0, 2).reshape(P, 28)
    ).astype(np.float16)
    cp32_base = np.zeros((P, 8), dtype=np.float32)
    cp32_base[:, 0:7] = b.astype(np.float32)[None, :]

    in_maps = []
    for k in range(NCORES):
        lo, hi = int(bounds[k]), int(bounds[k + 1])
        n = hi - lo
        xs = np.zeros((cap, F), dtype=np.float16)
        xs[:n] = x16[lo:hi]
        # chunk-contiguous, partition-major within each chunk
        xs = xs.reshape(ntiles, P, F)
        parts = [
            np.ascontiguousarray(xs[c0 : c0 + clen].transpose(1, 0, 2)).reshape(-1)
            for c0, clen in chunk_plan
        ]
        xs = np.concatenate(parts)
        blv = np.full((cap,), -1.0, dtype=np.float16)
        blv[:n] = (batch[lo:hi] - GPC * k).astype(np.float16)
        cp16 = np.empty((P, ntiles + GPC + P + 28), dtype=np.float16)
        cp16[:, 0:ntiles] = blv.reshape(ntiles, P).T
        cp16[:, ntiles : ntiles + GPC] = iota
        cp16[:, ntiles + GPC : ntiles + GPC + P] = np.eye(P, dtype=np.float16)
        cp16[:, ntiles + GPC + P :] = wtr
        cp32 = cp32_base.copy()
        cp32[:, 7] = inv_counts[GPC * k : GPC * (k + 1)]
        in_maps.append({"xs": xs, "cp16": cp16, "cp32": cp32})
    return in_maps


_last_result = None  # test harness can read exec_time_ns / trace from here


def kernel(x, edge_index, edge_attr, batch_size, W, b):
    from concourse import bass_utils

    global _last_result

    x16 = np.asarray(x, dtype=np.float16)
    batch = np.asarray(batch_size).astype(np.int64)
    W = np.asarray(W, dtype=np.float32)
    b = np.asarray(b, dtype=np.float32)

    if batch.size > 1 and np.any(np.diff(batch) < 0):
        # contiguous-shard logic needs sorted ids; reordering nodes does not
        # change per-graph sums
        order = np.argsort(batch, kind="stable")
        batch = batch[order]
        x16 = x16[order]

    counts = np.bincount(batch, minlength=G)
    inv_counts = (1.0 / np.maximum(counts, 1)).astype(np.float32)
    bounds = np.searchsorted(batch, np.arange(0, G + 1, GPC))
    max_rows = int(np.diff(bounds).max())
    ntiles = max(-(-max_rows // P), 1)

    nc = _get_compiled(ntiles)
    in_maps = _prep_in_maps(x16, batch, W, b, ntiles, bounds, inv_counts)

    res = bass_utils.run_bass_kernel_spmd(
        nc, in_maps, core_ids=list(range(NCORES))
    )
    _last_result = res

    # each core returns out [128, 7] for its graphs; assemble [1024, 7]
    out = np.concatenate(
        [np.asarray(res.results[k]["out"]) for k in range(NCORES)], axis=0
    )
    return np.ascontiguousarray(out.astype(np.float32))


# revision 71
# speedup vs baseline: 1.1052x; 1.0815x over previous
"""Trainium2 Bass kernel for MoGNN forward (global mean-pool + linear).

The model's conv outputs are discarded; the result depends only on x:
    pooled[g] = mean over nodes n with batch[n] == g of x[n]   # [1024, 512]
    out = pooled @ W.T + b                                     # [1024, 7]

batch ids are sorted, so nodes of each graph are contiguous. We shard by
GRAPHS: core k owns graphs [128k, 128k+128) and exactly the contiguous row
range of x belonging to them (padded to a tile multiple). No collectives.

Per 128-node tile, on device:
  - DVE builds an exact one-hot matrix oh[n, g] = (batch_local[n] == g);
    one tensor_tensor(is_equal) per DMA chunk via step-0 broadcast APs.
  - PE matmul (fp16 in, fp32 PSUM accumulate, full rate at N=512) does
    psum[128 graphs, 512 feats] += oh.T @ x_tile.
Epilogue (all fp16 for full-rate PE): PSUM -> SBUF with a per-graph 1/count
scale (mean pool), 4x PE transpose to feat-major (pipelined behind the
sliced scale), then 4 fp16 matmuls with pooled.T stationary and the W chunk
moving (N=7, fp32 PSUM), bias added via a partition-replicated fp32 tile;
each core writes out[128, 7] and the host concatenates to [1024, 7].

x is shipped as fp16 (11-bit effective mantissa; accumulation stays fp32 in
PSUM) — measured end-to-end relative error vs the fp32 reference ~2e-4,
comparable to the fp32r (tf32-like) matmul path while halving HBM traffic.
"""

import numpy as np

NCORES = 8
G = 1024            # total graphs
GPC = G // NCORES   # graphs per core = 128
F = 512             # feature dim
P = 128             # partition / node-tile size
CHUNK = 8           # node tiles per DMA chunk (1 MB fp16 transfers)

_compiled_cache = {}


def _chunk_plan(ntiles):
    """Chunk boundaries: small leading chunks so the PE pipeline starts early,
    CHUNK-tile steady state, and a small taper at the end so the PE finishes
    right behind the final DMA bytes."""
    head = [min(2, CHUNK), min(6, CHUNK)]
    tail = [min(2, CHUNK)]
    main_end = max(ntiles - sum(tail), 0)
    chunks = []
    t0 = 0
    for ramp in head:
        if t0 < main_end:
            clen = min(ramp, main_end - t0)
            chunks.append((t0, clen))
            t0 += clen
    while t0 < main_end:
        clen = min(CHUNK, main_end - t0)
        chunks.append((t0, clen))
        t0 += clen
    for ramp in tail:
        if t0 < ntiles:
            clen = min(ramp, ntiles - t0)
            chunks.append((t0, clen))
            t0 += clen
    while t0 < ntiles:
        clen = min(CHUNK, ntiles - t0)
        chunks.append((t0, clen))
        t0 += clen
    assert sum(c for _, c in chunks) == ntiles
    return chunks


def _build(ntiles):
    """Build + compile the per-core Bass kernel for a shard of `ntiles` node tiles."""
    from concourse import bacc, tile, mybir

    f32 = mybir.dt.float32
    f16 = mybir.dt.float16
    eq = mybir.AluOpType.is_equal
    mult = mybir.AluOpType.mult
    add = mybir.AluOpType.add

    nrows = ntiles * P
    chunks = _chunk_plan(ntiles)

    nc = bacc.Bacc(
        "TRN2",
        target_bir_lowering=False,
        debug=False,
        num_devices=NCORES,
    )

    # x shard laid out chunk-contiguous and partition-major inside each chunk:
    # for chunk (c0, clen), the DRAM block holds block[p, t, m] = x[(c0+t)*128+p, m]
    # so the whole chunk is one contiguous region and each partition reads one
    # contiguous multi-KB run
    x_d = nc.dram_tensor("xs", [nrows * F], f16, kind="ExternalInput")
    # constants packed into two tensors (one DMA each, on the scalar-engine
    # HWDGE ring so they don't block the x-chunk FIFO on the sync ring):
    # cp16 = [bl | iota], cp32 = [ident | wtr | b | icnt]
    # cp16 = [bl | iota | ident | wtr], cp32 = [b_replicated | icnt]
    cp16_d = nc.dram_tensor(
        "cp16", [P, ntiles + GPC + P + 28], f16, kind="ExternalInput"
    )
    cp32_d = nc.dram_tensor("cp32", [P, 8], f32, kind="ExternalInput")
    out_d = nc.dram_tensor("out", [GPC, 7], f32, kind="ExternalOutput")

    with tile.TileContext(nc) as tc:
        with (
            tc.tile_pool(name="const", bufs=1) as constp,
            tc.tile_pool(name="xin", bufs=4) as xp,
            tc.tile_pool(name="oh", bufs=4) as ohp,
            tc.tile_pool(name="acc", bufs=1, space="PSUM") as accp,
            tc.tile_pool(name="tps", bufs=2, space="PSUM") as tpsp,
            tc.tile_pool(name="outp", bufs=1, space="PSUM") as outpp,
            tc.tile_pool(name="sb", bufs=2) as sbp,
        ):
            cp16_t = constp.tile([P, ntiles + GPC + P + 28], f16)
            nc.scalar.dma_start(cp16_t[:], cp16_d.ap())
            cp32_t = constp.tile([P, 8], f32)
            nc.scalar.dma_start(cp32_t[:], cp32_d.ap())
            bl_t = cp16_t[:, 0:ntiles]
            iota_t = cp16_t[:, ntiles : ntiles + GPC]
            ident_t = cp16_t[:, ntiles + GPC : ntiles + GPC + P]
            wtr_t = cp16_t[:, ntiles + GPC + P : ntiles + GPC + P + 28]
            brep_t = cp32_t[:, 0:7]
            icnt_t = cp32_t[:, 7:8]

            acc = accp.tile([GPC, F], f32)
            x_flat = x_d.ap()

            iota_rep = iota_t.rearrange("p (a g) -> p a g", a=1)
            t = 0
            for c0, clen in chunks:
                xt = xp.tile([P, CHUNK, F], f16, tag="xt")
                chunk_ap = x_flat[c0 * P * F : (c0 + clen) * P * F].rearrange(
                    "(p t m) -> p t m", p=P, m=F
                )
                nc.sync.dma_start(xt[:, :clen, :], chunk_ap)
                # one-hot for the whole chunk in one DVE op via broadcast APs:
                # oh[p, n, g] = (iota[g] == bl[p, c0+n])
                oh = ohp.tile([P, CHUNK, GPC], f16, tag="oh")
                nc.vector.tensor_tensor(
                    oh[:, :clen, :],
                    iota_rep.broadcast_to([P, clen, GPC]),
                    bl_t[:, c0 : c0 + clen]
                    .rearrange("p (n a) -> p n a", a=1)
                    .broadcast_to([P, clen, GPC]),
                    op=eq,
                )
                for n in range(clen):
                    nc.tensor.matmul(
                        acc[:],
                        oh[:, n, :],
                        xt[:, n, :],
                        start=(t == 0),
                        stop=(t == ntiles - 1),
                    )
                    t += 1

            # pooled = acc * (1/count[g]) cast to fp16, sliced so the (fp16,
            # full-rate) transposes pipeline behind the scale copies; then the
            # classifier with pooled.T as stationary (moving is W [128, 7], N=7)
            pooled = sbp.tile([GPC, F], f16)
            ptall = sbp.tile([P, 4, P], f16)
            for j in range(4):
                sl = slice(j * P, (j + 1) * P)
                nc.vector.tensor_scalar(
                    pooled[:, sl], acc[:, sl], icnt_t, None, op0=mult
                )
                tp = tpsp.tile([P, P], f16)
                nc.tensor.transpose(tp[:], pooled[:, sl], ident_t)
                nc.vector.tensor_copy(ptall[:, j, :], tp[:])

            out_ps = outpp.tile([GPC, 7], f32)
            for j in range(4):
                nc.tensor.matmul(
                    out_ps[:],
                    ptall[:, j, :],
                    wtr_t[:, j * 7 : (j + 1) * 7],
                    start=(j == 0),
                    stop=(j == 3),
                )

            out_sb = sbp.tile([GPC, 7], f32)
            nc.vector.tensor_tensor(out_sb[:], out_ps[:], brep_t, op=add)
            nc.sync.dma_start(out_d.ap(), out_sb[:])

    nc.compile()
    return nc


def _get_compiled(ntiles):
    if ntiles not in _compiled_cache:
        _compiled_cache[ntiles] = _build(ntiles)
    return _compiled_cache[ntiles]


def _prep_in_maps(x16, batch, W, b, ntiles, bounds, inv_counts):
    cap = ntiles * P
    chunk_plan = _chunk_plan(ntiles)
    iota = np.tile(np.arange(GPC, dtype=np.float16)[None, :], (P, 1))
    # wtr[p, c*7+j] = W.T[c*128+p, j]
    wtr = np.ascontiguousarray(
        W.T.reshape(4, P, 7).transpose(1, 0, 2).reshape(P, 28)
    ).astype(np.float16)
    cp32_base = np.zeros((P, 8), dtype=np.float32)
    cp32_base[:, 0:7] = b.astype(np.float32)[None, :]

    in_maps = []
    for k in range(NCORES):
        lo, hi = int(bounds[k]), int(bounds[k + 1])
        n = hi - lo
        xs = np.zeros((cap, F), dtype=np.float16)
        xs[:n] = x16[lo:hi]
        # chunk-contiguous, partition-major within each chunk
        xs = xs.reshape(ntiles, P, F)
        parts = [
            np.ascontiguousarray(xs[c0 : c0 + clen].transpose(1, 0, 2)).reshape(-1)
            for c0, clen in chunk_plan
        ]
        xs = np.concatenate(parts)
        blv = np.full((cap,), -1.0, dtype=np.float16)
        blv[:n] = (batch[lo:hi] - GPC * k).astype(np.float16)
        cp16 = np.empty((P, ntiles + GPC + P + 28), dtype=np.float16)
        cp16[:, 0:ntiles] = blv.reshape(ntiles, P).T
        cp16[:, ntiles : ntiles + GPC] = iota
        cp16[:, ntiles + GPC : ntiles + GPC + P] = np.eye(P, dtype=np.float16)
        cp16[:, ntiles + GPC + P :] = wtr
        cp32 = cp32_base.copy()
        cp32[:, 7] = inv_counts[GPC * k : GPC * (k + 1)]
        in_maps.append({"xs": xs, "cp16": cp16, "cp32": cp32})
    return in_maps


_last_result = None  # test harness can read exec_time_ns / trace from here


def kernel(x, edge_index, edge_attr, batch_size, W, b):
    from concourse import bass_utils

    global _last_result

    x16 = np.asarray(x, dtype=np.float16)
    batch = np.asarray(batch_size).astype(np.int64)
    W = np.asarray(W, dtype=np.float32)
    b = np.asarray(b, dtype=np.float32)

    if batch.size > 1 and np.any(np.diff(batch) < 0):
        # contiguous-shard logic needs sorted ids; reordering nodes does not
        # change per-graph sums
        order = np.argsort(batch, kind="stable")
        batch = batch[order]
        x16 = x16[order]

    counts = np.bincount(batch, minlength=G)
    inv_counts = (1.0 / np.maximum(counts, 1)).astype(np.float32)
    bounds = np.searchsorted(batch, np.arange(0, G + 1, GPC))
    max_rows = int(np.diff(bounds).max())
    ntiles = max(-(-max_rows // P), 1)

    nc = _get_compiled(ntiles)
    in_maps = _prep_in_maps(x16, batch, W, b, ntiles, bounds, inv_counts)

    res = bass_utils.run_bass_kernel_spmd(
        nc, in_maps, core_ids=list(range(NCORES))
    )
    _last_result = res

    # each core returns out [128, 7] for its graphs; assemble [1024, 7]
    out = np.concatenate(
        [np.asarray(res.results[k]["out"]) for k in range(NCORES)], axis=0
    )
    return np.ascontiguousarray(out.astype(np.float32))
